# revision 48
# baseline (speedup 1.0000x reference)
"""MoE (top-2 of 8 experts, SwiGLU) Trainium2 kernel.

Expert-parallel over 8 NeuronCores: the host computes the tiny gate
(tanh(x@Wg1)@Wg2, ~0.07% of total FLOPs) and performs the all-to-all
token dispatch (gather per expert); each core runs the SwiGLU expert
FFN for its expert's tokens on device; the host combines the two
expert contributions per token.

Shapes (hardcoded): B=2, S=2048, D=2048, E=8, H=704, K=2.
"""

import sys

sys.path.insert(0, "/opt/trn_rl_repo")

import numpy as np
import ml_dtypes

B, S, D, E, H, K = 2, 2048, 2048, 8, 704, 2
T = B * S
BALANCE_W = 0.01
N_CORES = 8
P = 128
TOK_TILE = 448  # set per-dtype below
KC = D // P  # 16 contraction chunks of 128 over D
HC = (H + P - 1) // P  # 6 chunks over H (5 full + 64)

DT = "bf16"  # "bf16" or "f32r"
TOK_TILE = 448 if DT == "bf16" else 336
NWARM = 150  # HAM warm-up matmuls (N=128 bf16, ~61ns warm)

_RUNNERS = {}


def _split_excess_waits(nc, mybir, bass_rust):
    # walrus on this image caps sync waits at 1 per instruction (2 for
    # EventSemaphore); Tile's exit path can stuff more into the final
    # drain. Split the excess onto extra Drain instructions.
    for fn in nc.m.functions:
        for bb in fn.blocks:
            il = bb.instructions
            fixes = []
            for idx, inst in enumerate(il):
                si = inst.sync_info
                if si is None:
                    continue
                cap = 2 if inst.opcode == "EventSemaphore" else 1
                if len(si.on_wait) > cap:
                    fixes.append((idx, inst, cap))
            for idx, inst, cap in reversed(fixes):
                waits = list(inst.sync_info.on_wait)
                extra = waits[cap:]
                new_insts = []
                for k, w in enumerate(extra):
                    d = mybir.InstDrain(
                        name=f"{inst.name}-waitsplit-{k}", ins=[], outs=[]
                    )
                    d.engine = inst.engine
                    d.sync_info = bass_rust.SyncInfo(on_wait=[w], on_update=[])
                    new_insts.append(d)
                inst.sync_info = bass_rust.SyncInfo(
                    on_wait=waits[:cap], on_update=list(inst.sync_info.on_update)
                )
                il[idx:idx] = new_insts


def _build(C, dt_name):
    """Build the per-core expert-FFN Bass program for capacity C."""
    import concourse.bass as bass
    import concourse.mybir as mybir
    import concourse.tile as tile
    import bass_rust

    f32 = mybir.dt.float32
    if dt_name == "bf16":
        dt_io = mybir.dt.bfloat16
    else:
        # float32r storage: 4-byte fp32 bits, matmul runs the fast fp32r
        # path; compute producers round on write, DMA data is used as-is
        dt_io = mybir.dt.float32r

    HF = H // P  # 5 full H chunks; the 64-wide g/u tails are packed together
    HT = H - HF * P  # 64-wide tail
    HB = HF * P
    nc = bass.Bass("TRN2")
    # host supplies everything partition-major so each tensor loads with a
    # single large DMA (few issues, full SDMA fan-out)
    xt_d = nc.dram_tensor("xt", [P, KC, C], dt_io, kind="ExternalInput")
    # all weights concatenated per partition: [wg | wu | wgu | wd]
    W_G, W_U, W_GU, W_D = 0, KC * HB, 2 * KC * HB, 2 * KC * HB + KC * P
    W_TOT = W_D + HC * D
    w_d = nc.dram_tensor("w", [P, W_TOT], dt_io, kind="ExternalInput")
    sc_d = nc.dram_tensor("sc", [1, C], f32, kind="ExternalInput")
    yt_d = nc.dram_tensor("yt", [KC, P, C], f32, kind="ExternalOutput")

    n_tiles = C // TOK_TILE
    silu_t = mybir.ActivationFunctionType.Silu

    def mm(out, lhsT, rhs, start, stop):
        nc.tensor.matmul(out, lhsT, rhs, start=start, stop=stop)

    xbufs = 4 if dt_name == "bf16" else 2
    obufs = 4 if dt_name == "bf16" else 2
    scbufs = 5 if dt_name == "bf16" else 3
    tbufs = 2 if dt_name == "bf16" else 1
    with tile.TileContext(nc) as tc:
        with tc.tile_pool(name="wpool", bufs=1) as wpool, \
             tc.tile_pool(name="xpool", bufs=xbufs) as xpool, \
             tc.tile_pool(name="hpool", bufs=2) as hpool, \
             tc.tile_pool(name="spool", bufs=2) as spool, \
             tc.tile_pool(name="opool", bufs=obufs) as opool, \
             tc.tile_pool(name="pg", bufs=3, space="PSUM") as pg_pool, \
             tc.tile_pool(name="pu", bufs=3, space="PSUM") as pu_pool, \
             tc.tile_pool(name="py", bufs=2, space="PSUM") as py_pool:

            # HAM warm-up: throwaway matmuls with no DMA dependencies keep
            # the PE busy (and the clock un-gated) while the first weights
            # and activations stream in. The PSUM accumulator is never read.
            wdt = mybir.dt.bfloat16  # warm-up dummies are always bf16
            warm_w = wpool.tile([P, P], wdt, name="warm_w")
            nc.vector.memset(warm_w[:], 0.0)
            pwarm = pg_pool.tile([P, TOK_TILE], f32, tag="pg")
            for i in range(NWARM):
                nc.tensor.matmul(pwarm[:, :P], warm_w[:], warm_w[:],
                                 start=True, stop=True)

            KQ = KC // 4

            def alloc_tile():
                xt_sb = [
                    xpool.tile([P, KQ, TOK_TILE], dt_io, name=f"xtq{q}", tag=f"xtq{q}")
                    for q in range(4)
                ]
                sc_sb = spool.tile([P, TOK_TILE], f32, tag="sc", bufs=scbufs)
                return xt_sb, sc_sb

            def dma_quarter(xt_sc, j, q):
                js = bass.ts(j, TOK_TILE)
                nc.sync.dma_start(
                    xt_sc[0][q][:], xt_d[:, q * KQ : (q + 1) * KQ, js]
                )

            def dma_sc(xt_sc, j):
                js = bass.ts(j, TOK_TILE)
                nc.scalar.dma_start(
                    xt_sc[1][:, :], sc_d[0:1, js].to_broadcast((P, TOK_TILE))
                )

            def load_tile(j):
                xt_sc = alloc_tile()
                for q in range(4):
                    dma_quarter(xt_sc, j, q)
                dma_sc(xt_sc, j)
                return xt_sc

            def xtl(xt_sb, kc):
                return xt_sb[kc // KQ][:, kc % KQ, :]

            # resident weights, split in quarters for progressive availability
            wg_sb = [wpool.tile([P, KQ, HB], dt_io, name=f"wg{i}", tag=f"wg{i}")
                     for i in range(4)]
            wu_sb = [wpool.tile([P, KQ, HB], dt_io, name=f"wu{i}", tag=f"wu{i}")
                     for i in range(4)]
            wgu_sb = wpool.tile([P, KC, P], dt_io, name="wgu")
            wd_sb = wpool.tile([P, HC, D], dt_io, name="wd")

            def wgl(kc):  # lhsT slice helpers
                return wg_sb[kc // KQ][:, kc % KQ, :]

            def wul(kc):
                return wu_sb[kc // KQ][:, kc % KQ, :]

            # DMA emission order = consumption order; xt/wg quarters
            # interleaved so the first matmuls wait on <1MB
            PREFETCH = min(n_tiles, xbufs)
            def wslice(off, ln):
                return w_d[:, off : off + ln]

            QW = KQ * HB  # cols per wg/wu quarter
            xts = {}
            xts[0] = alloc_tile()
            dma_quarter(xts[0], 0, 0)
            nc.sync.dma_start(wg_sb[0][:].rearrange("p a b -> p (a b)"),
                              wslice(W_G, QW))
            for q in range(1, 4):
                dma_quarter(xts[0], 0, q)
                nc.sync.dma_start(wg_sb[q][:].rearrange("p a b -> p (a b)"),
                                  wslice(W_G + q * QW, QW))
            dma_sc(xts[0], 0)
            for q in range(4):
                nc.sync.dma_start(wu_sb[q][:].rearrange("p a b -> p (a b)"),
                                  wslice(W_U + q * QW, QW))
            nc.sync.dma_start(wgu_sb[:].rearrange("p a b -> p (a b)"),
                              wslice(W_GU, KC * P))
            if n_tiles > 1:
                xts[1] = load_tile(1)
            nc.sync.dma_start(wd_sb[:].rearrange("p a b -> p (a b)"),
                              wslice(W_D, HC * D))
            for j in range(2, PREFETCH):
                xts[j] = load_tile(j)

            def fuse_hc(hc, pg, pu, h_sb, hw=P):
                silu_sb = spool.tile([P, TOK_TILE], f32, tag="silu")
                nc.scalar.activation(silu_sb[:hw, :], pg[:hw, :], silu_t)
                nc.vector.tensor_mul(h_sb[:hw, hc, :], silu_sb[:hw, :], pu[:hw, :])

            def tail_hc(pgu, h_sb):
                # packed tail: partitions [0:HT] hold g-tail, [HT:2*HT] u-tail.
                # Engines are lane-aligned, so copy the u half to SBUF then
                # DMA-shift it down to partitions [0:HT].
                u_tmp = spool.tile([P, TOK_TILE], f32, tag="utmp", bufs=tbufs)
                nc.scalar.copy(u_tmp[HT : 2 * HT, :], pgu[HT : 2 * HT, :])
                u_sb = spool.tile([P, TOK_TILE], f32, tag="utail", bufs=tbufs)
                nc.scalar.dma_start(u_sb[:HT, :], u_tmp[HT : 2 * HT, :])
                silu_sb = spool.tile([P, TOK_TILE], f32, tag="silu")
                nc.scalar.activation(silu_sb[:HT, :], pgu[:HT, :], silu_t)
                nc.vector.tensor_mul(
                    h_sb[:HT, HF, :], silu_sb[:HT, :], u_sb[:HT, :]
                )

            def gate_up(j, xt_sc):
                xt_sb, sc_sb = xt_sc
                h_sb = hpool.tile([P, HC, TOK_TILE], dt_io, tag="h")
                if H % P and dt_name == "bf16":
                    # zero h padding partitions: down matmul contracts K=128
                    # (keeps FWL on; for f32r FWL is off anyway so the down
                    # matmul just uses K=64 for the tail chunk)
                    nc.vector.memset(h_sb[H % P :, HC - 1, :], 0.0)
                if j == 0:
                    # kc-outer over groups of 3 banks: consume weight halves
                    # as they arrive off DMA during the ramp
                    for hcs in ([0, 1, 2], [3, 4]):
                        pg = {hc: pg_pool.tile([P, TOK_TILE], f32, name=f"pg{hc}", tag="pg") for hc in hcs}
                        pu = {hc: pu_pool.tile([P, TOK_TILE], f32, name=f"pu{hc}", tag="pu") for hc in hcs}
                        for kc in range(KC):
                            for hc in hcs:
                                hs = bass.ts(hc, P)
                                mm(pg[hc][:], wgl(kc)[:, hs], xtl(xt_sb, kc),
                                   start=(kc == 0), stop=(kc == KC - 1))
                        for kc in range(KC):
                            for hc in hcs:
                                hs = bass.ts(hc, P)
                                mm(pu[hc][:], wul(kc)[:, hs], xtl(xt_sb, kc),
                                   start=(kc == 0), stop=(kc == KC - 1))
                        for hc in hcs:
                            fuse_hc(hc, pg[hc], pu[hc], h_sb)
                else:
                    for hc in range(HF):
                        hs = bass.ts(hc, P)
                        pg = pg_pool.tile([P, TOK_TILE], f32, tag="pg")
                        pu = pu_pool.tile([P, TOK_TILE], f32, tag="pu")
                        for kc in range(KC):
                            mm(pg[:], wgl(kc)[:, hs], xtl(xt_sb, kc),
                               start=(kc == 0), stop=(kc == KC - 1))
                        for kc in range(KC):
                            mm(pu[:], wul(kc)[:, hs], xtl(xt_sb, kc),
                               start=(kc == 0), stop=(kc == KC - 1))
                        fuse_hc(hc, pg, pu, h_sb)
                # packed g/u tail: one matmul column block for both
                pgu = pg_pool.tile([P, TOK_TILE], f32, tag="pg")
                for kc in range(KC):
                    mm(pgu[:], wgu_sb[:, kc, :], xtl(xt_sb, kc),
                       start=(kc == 0), stop=(kc == KC - 1))
                tail_hc(pgu, h_sb)
                return h_sb, sc_sb

            def down(j, h_sc):
                h_sb, sc_sb = h_sc
                js = bass.ts(j, TOK_TILE)
                for dc in range(KC):
                    psum_y = py_pool.tile([P, TOK_TILE], f32, tag="py")
                    ds_ = bass.ts(dc, P)
                    for hc in range(HC):
                        kw = P if dt_name == "bf16" else min(P, H - hc * P)
                        mm(psum_y, wd_sb[:kw, hc, ds_], h_sb[:kw, hc, :],
                           start=(hc == 0), stop=(hc == HC - 1))
                    yt_sb = opool.tile([P, TOK_TILE], f32, tag="yt")
                    nc.vector.tensor_mul(yt_sb, psum_y, sc_sb)
                    # outputs go out on the scalar engine's HWDGE ring so
                    # they never queue behind input streaming
                    nc.scalar.dma_start(yt_d[dc][:, js], yt_sb[:])

            # software pipeline: down(j) is emitted after gate_up(j+1) so the
            # PE never waits on the DVE h-tiles
            h_prev = gate_up(0, xts[0])
            for j in range(1, n_tiles):
                h_cur = gate_up(j, xts[j] if j in xts else load_tile(j))
                down(j - 1, h_prev)
                h_prev = h_cur
            down(n_tiles - 1, h_prev)

    _split_excess_waits(nc, mybir, bass_rust)
    return nc


def _get_runner(C, dt_name):
    key = (C, dt_name)
    if key not in _RUNNERS:
        _RUNNERS[key] = _build(C, dt_name)
    return _RUNNERS[key]


def _routing(xt, Wg1, Wg2):
    logits = np.tanh(xt @ Wg1) @ Wg2  # [T, E] fp32
    order = np.argsort(-logits, axis=1, kind="stable")[:, :K]  # top-2, ties->low idx
    top_v = np.take_along_axis(logits, order, axis=1)
    # softmax over the selected 2
    m = top_v.max(axis=1, keepdims=True)
    ex = np.exp(top_v - m)
    top_s = ex / ex.sum(axis=1, keepdims=True)  # [T, 2]
    scores = np.zeros((T, E), dtype=np.float32)
    np.put_along_axis(scores, order, top_s.astype(np.float32), axis=1)
    return scores


def _cv_sq(v):
    v = v.astype(np.float64)
    return v.var(ddof=1) / (v.mean() ** 2 + 1e-10)


def _prepare(x, Wg1, Wg2, W_gate, W_up, W_down):
    """Routing + dispatch: returns (in_maps, idxs, C, balance_loss)."""
    x = np.asarray(x, dtype=np.float32)
    xt = x.reshape(T, D)
    scores = _routing(xt, np.asarray(Wg1, np.float32), np.asarray(Wg2, np.float32))

    importance = scores.sum(axis=0)
    load = (scores > 0).sum(axis=0).astype(np.float64)
    balance_loss = np.float32(BALANCE_W * (_cv_sq(importance) + _cv_sq(load)))

    idxs = [np.nonzero(scores[:, e] > 0)[0] for e in range(E)]
    max_cnt = max(len(i) for i in idxs)
    C = max(TOK_TILE, ((max_cnt + TOK_TILE - 1) // TOK_TILE) * TOK_TILE)

    np_io = ml_dtypes.bfloat16 if DT == "bf16" else np.float32
    W_gate = np.asarray(W_gate, np.float32)
    W_up = np.asarray(W_up, np.float32)
    W_down = np.asarray(W_down, np.float32)
    HF = H // P
    HB = HF * P

    in_maps = []
    for e in range(E):
        idx = idxs[e]
        # partition-major layouts: [P, KC, ...] so each tensor is one DMA
        xt_sel = np.zeros((KC, P, C), dtype=np_io)
        xt_sel.reshape(D, C)[:, : len(idx)] = xt[idx].T
        sc = np.zeros((1, C), dtype=np.float32)
        sc[0, : len(idx)] = scores[idx, e]
        wg3 = W_gate[e].astype(np_io).reshape(KC, P, H)
        wu3 = W_up[e].astype(np_io).reshape(KC, P, H)
        wd5 = np.zeros((HC, P, D), dtype=np_io)
        wd5.reshape(HC * P, D)[:H] = W_down[e].astype(np_io)
        wd4 = np.ascontiguousarray(wd5.transpose(1, 0, 2))
        w_all = np.concatenate(
            [
                wg3[:, :, :HB].transpose(1, 0, 2).reshape(P, -1),
                wu3[:, :, :HB].transpose(1, 0, 2).reshape(P, -1),
                np.concatenate([wg3[:, :, HB:], wu3[:, :, HB:]], axis=2)
                .transpose(1, 0, 2)
                .reshape(P, -1),
                wd4.reshape(P, -1),
            ],
            axis=1,
        )
        in_maps.append(
            {
                "xt": np.ascontiguousarray(xt_sel.transpose(1, 0, 2)),
                "w": np.ascontiguousarray(w_all),
                "sc": sc,
            }
        )
    return in_maps, idxs, C, balance_loss


def kernel(x, Wg1, Wg2, W_gate, W_up, W_down, num_selects):
    from concourse.bass_utils import run_bass_kernel_spmd

    assert int(num_selects) == K
    in_maps, idxs, C, balance_loss = _prepare(x, Wg1, Wg2, W_gate, W_up, W_down)

    nc = _get_runner(C, DT)
    res = run_bass_kernel_spmd(nc, in_maps, core_ids=list(range(N_CORES)))

    y = np.zeros((T, D), dtype=np.float32)
    for e in range(E):
        idx = idxs[e]
        yt = res.results[e]["yt"].reshape(D, C)
        y[idx] += yt[:, : len(idx)].T

    return y.reshape(np.asarray(x).shape[:-1] + (D,)), balance_loss


# revision 49
# speedup vs baseline: 1.0053x; 1.0053x over previous
"""MoE (top-2 of 8 experts, SwiGLU) Trainium2 kernel.

Expert-parallel over 8 NeuronCores: the host computes the tiny gate
(tanh(x@Wg1)@Wg2, ~0.07% of total FLOPs) and performs the all-to-all
token dispatch (gather per expert); each core runs the SwiGLU expert
FFN for its expert's tokens on device; the host combines the two
expert contributions per token.

Shapes (hardcoded): B=2, S=2048, D=2048, E=8, H=704, K=2.
"""

import sys

sys.path.insert(0, "/opt/trn_rl_repo")

import numpy as np
import ml_dtypes

B, S, D, E, H, K = 2, 2048, 2048, 8, 704, 2
T = B * S
BALANCE_W = 0.01
N_CORES = 8
P = 128
TOK_TILE = 448  # set per-dtype below
KC = D // P  # 16 contraction chunks of 128 over D
HC = (H + P - 1) // P  # 6 chunks over H (5 full + 64)

DT = "bf16"  # "bf16" or "f32r"
TOK_TILE = 448 if DT == "bf16" else 336
NWARM = 135  # HAM warm-up matmuls (N=128 bf16, ~61ns warm)

_RUNNERS = {}


def _split_excess_waits(nc, mybir, bass_rust):
    # walrus on this image caps sync waits at 1 per instruction (2 for
    # EventSemaphore); Tile's exit path can stuff more into the final
    # drain. Split the excess onto extra Drain instructions.
    for fn in nc.m.functions:
        for bb in fn.blocks:
            il = bb.instructions
            fixes = []
            for idx, inst in enumerate(il):
                si = inst.sync_info
                if si is None:
                    continue
                cap = 2 if inst.opcode == "EventSemaphore" else 1
                if len(si.on_wait) > cap:
                    fixes.append((idx, inst, cap))
            for idx, inst, cap in reversed(fixes):
                waits = list(inst.sync_info.on_wait)
                extra = waits[cap:]
                new_insts = []
                for k, w in enumerate(extra):
                    d = mybir.InstDrain(
                        name=f"{inst.name}-waitsplit-{k}", ins=[], outs=[]
                    )
                    d.engine = inst.engine
                    d.sync_info = bass_rust.SyncInfo(on_wait=[w], on_update=[])
                    new_insts.append(d)
                inst.sync_info = bass_rust.SyncInfo(
                    on_wait=waits[:cap], on_update=list(inst.sync_info.on_update)
                )
                il[idx:idx] = new_insts


def _build(C, dt_name):
    """Build the per-core expert-FFN Bass program for capacity C."""
    import concourse.bass as bass
    import concourse.mybir as mybir
    import concourse.tile as tile
    import bass_rust

    f32 = mybir.dt.float32
    if dt_name == "bf16":
        dt_io = mybir.dt.bfloat16
    else:
        # float32r storage: 4-byte fp32 bits, matmul runs the fast fp32r
        # path; compute producers round on write, DMA data is used as-is
        dt_io = mybir.dt.float32r

    HF = H // P  # 5 full H chunks; the 64-wide g/u tails are packed together
    HT = H - HF * P  # 64-wide tail
    HB = HF * P
    nc = bass.Bass("TRN2")
    # host supplies everything partition-major so each tensor loads with a
    # single large DMA (few issues, full SDMA fan-out)
    xt_d = nc.dram_tensor("xt", [P, KC, C], dt_io, kind="ExternalInput")
    # all weights concatenated per partition: [wg | wu | wgu | wd]
    W_G, W_U, W_GU, W_D = 0, KC * HB, 2 * KC * HB, 2 * KC * HB + KC * P
    W_TOT = W_D + HC * D
    w_d = nc.dram_tensor("w", [P, W_TOT], dt_io, kind="ExternalInput")
    sc_d = nc.dram_tensor("sc", [1, C], f32, kind="ExternalInput")
    yt_d = nc.dram_tensor("yt", [KC, P, C], f32, kind="ExternalOutput")

    n_tiles = C // TOK_TILE
    silu_t = mybir.ActivationFunctionType.Silu

    def mm(out, lhsT, rhs, start, stop):
        nc.tensor.matmul(out, lhsT, rhs, start=start, stop=stop)

    xbufs = 4 if dt_name == "bf16" else 2
    obufs = 4 if dt_name == "bf16" else 2
    scbufs = 5 if dt_name == "bf16" else 3
    tbufs = 2 if dt_name == "bf16" else 1
    with tile.TileContext(nc) as tc:
        with tc.tile_pool(name="wpool", bufs=1) as wpool, \
             tc.tile_pool(name="xpool", bufs=xbufs) as xpool, \
             tc.tile_pool(name="hpool", bufs=2) as hpool, \
             tc.tile_pool(name="spool", bufs=2) as spool, \
             tc.tile_pool(name="opool", bufs=obufs) as opool, \
             tc.tile_pool(name="pg", bufs=3, space="PSUM") as pg_pool, \
             tc.tile_pool(name="pu", bufs=3, space="PSUM") as pu_pool, \
             tc.tile_pool(name="py", bufs=2, space="PSUM") as py_pool:

            # HAM warm-up: throwaway matmuls with no DMA dependencies keep
            # the PE busy (and the clock un-gated) while the first weights
            # and activations stream in. The PSUM accumulator is never read.
            wdt = mybir.dt.bfloat16  # warm-up dummies are always bf16
            warm_w = wpool.tile([P, P], wdt, name="warm_w")
            nc.vector.memset(warm_w[:], 0.0)
            pwarm = pg_pool.tile([P, TOK_TILE], f32, tag="pg")
            for i in range(NWARM):
                nc.tensor.matmul(pwarm[:, :P], warm_w[:], warm_w[:],
                                 start=True, stop=True)

            KQ = KC // 4

            def alloc_tile():
                xt_sb = [
                    xpool.tile([P, KQ, TOK_TILE], dt_io, name=f"xtq{q}", tag=f"xtq{q}")
                    for q in range(4)
                ]
                sc_sb = spool.tile([P, TOK_TILE], f32, tag="sc", bufs=scbufs)
                return xt_sb, sc_sb

            def dma_quarter(xt_sc, j, q):
                js = bass.ts(j, TOK_TILE)
                nc.sync.dma_start(
                    xt_sc[0][q][:], xt_d[:, q * KQ : (q + 1) * KQ, js]
                )

            def dma_sc(xt_sc, j):
                js = bass.ts(j, TOK_TILE)
                nc.scalar.dma_start(
                    xt_sc[1][:, :], sc_d[0:1, js].to_broadcast((P, TOK_TILE))
                )

            def load_tile(j):
                xt_sc = alloc_tile()
                for q in range(4):
                    dma_quarter(xt_sc, j, q)
                dma_sc(xt_sc, j)
                return xt_sc

            def xtl(xt_sb, kc):
                return xt_sb[kc // KQ][:, kc % KQ, :]

            # resident weights, split in quarters for progressive availability
            wg_sb = [wpool.tile([P, KQ, HB], dt_io, name=f"wg{i}", tag=f"wg{i}")
                     for i in range(4)]
            wu_sb = [wpool.tile([P, KQ, HB], dt_io, name=f"wu{i}", tag=f"wu{i}")
                     for i in range(4)]
            wgu_sb = wpool.tile([P, KC, P], dt_io, name="wgu")
            wd_sb = wpool.tile([P, HC, D], dt_io, name="wd")

            def wgl(kc):  # lhsT slice helpers
                return wg_sb[kc // KQ][:, kc % KQ, :]

            def wul(kc):
                return wu_sb[kc // KQ][:, kc % KQ, :]

            # DMA emission order = consumption order; xt/wg quarters
            # interleaved so the first matmuls wait on <1MB
            PREFETCH = min(n_tiles, xbufs)
            def wslice(off, ln):
                return w_d[:, off : off + ln]

            QW = KQ * HB  # cols per wg/wu quarter
            xts = {}
            xts[0] = alloc_tile()
            dma_quarter(xts[0], 0, 0)
            nc.sync.dma_start(wg_sb[0][:].rearrange("p a b -> p (a b)"),
                              wslice(W_G, QW))
            for q in range(1, 4):
                dma_quarter(xts[0], 0, q)
                nc.sync.dma_start(wg_sb[q][:].rearrange("p a b -> p (a b)"),
                                  wslice(W_G + q * QW, QW))
            dma_sc(xts[0], 0)
            for q in range(4):
                nc.sync.dma_start(wu_sb[q][:].rearrange("p a b -> p (a b)"),
                                  wslice(W_U + q * QW, QW))
            nc.sync.dma_start(wgu_sb[:].rearrange("p a b -> p (a b)"),
                              wslice(W_GU, KC * P))
            if n_tiles > 1:
                xts[1] = load_tile(1)
            nc.sync.dma_start(wd_sb[:].rearrange("p a b -> p (a b)"),
                              wslice(W_D, HC * D))
            for j in range(2, PREFETCH):
                xts[j] = load_tile(j)

            def fuse_hc(hc, pg, pu, h_sb, hw=P):
                silu_sb = spool.tile([P, TOK_TILE], f32, tag="silu")
                nc.scalar.activation(silu_sb[:hw, :], pg[:hw, :], silu_t)
                nc.vector.tensor_mul(h_sb[:hw, hc, :], silu_sb[:hw, :], pu[:hw, :])

            def tail_hc(pgu, h_sb):
                # packed tail: partitions [0:HT] hold g-tail, [HT:2*HT] u-tail.
                # Engines are lane-aligned, so copy the u half to SBUF then
                # DMA-shift it down to partitions [0:HT].
                u_tmp = spool.tile([P, TOK_TILE], f32, tag="utmp", bufs=tbufs)
                nc.scalar.copy(u_tmp[HT : 2 * HT, :], pgu[HT : 2 * HT, :])
                u_sb = spool.tile([P, TOK_TILE], f32, tag="utail", bufs=tbufs)
                nc.scalar.dma_start(u_sb[:HT, :], u_tmp[HT : 2 * HT, :])
                silu_sb = spool.tile([P, TOK_TILE], f32, tag="silu")
                nc.scalar.activation(silu_sb[:HT, :], pgu[:HT, :], silu_t)
                nc.vector.tensor_mul(
                    h_sb[:HT, HF, :], silu_sb[:HT, :], u_sb[:HT, :]
                )

            def gate_up(j, xt_sc):
                xt_sb, sc_sb = xt_sc
                h_sb = hpool.tile([P, HC, TOK_TILE], dt_io, tag="h")
                if H % P and dt_name == "bf16":
                    # zero h padding partitions: down matmul contracts K=128
                    # (keeps FWL on; for f32r FWL is off anyway so the down
                    # matmul just uses K=64 for the tail chunk)
                    nc.vector.memset(h_sb[H % P :, HC - 1, :], 0.0)
                if j == 0:
                    # kc-outer over groups of 3 banks: consume weight halves
                    # as they arrive off DMA during the ramp
                    for hcs in ([0, 1, 2], [3, 4]):
                        pg = {hc: pg_pool.tile([P, TOK_TILE], f32, name=f"pg{hc}", tag="pg") for hc in hcs}
                        pu = {hc: pu_pool.tile([P, TOK_TILE], f32, name=f"pu{hc}", tag="pu") for hc in hcs}
                        for kc in range(KC):
                            for hc in hcs:
                                hs = bass.ts(hc, P)
                                mm(pg[hc][:], wgl(kc)[:, hs], xtl(xt_sb, kc),
                                   start=(kc == 0), stop=(kc == KC - 1))
                        for kc in range(KC):
                            for hc in hcs:
                                hs = bass.ts(hc, P)
                                mm(pu[hc][:], wul(kc)[:, hs], xtl(xt_sb, kc),
                                   start=(kc == 0), stop=(kc == KC - 1))
                        for hc in hcs:
                            fuse_hc(hc, pg[hc], pu[hc], h_sb)
                else:
                    for hc in range(HF):
                        hs = bass.ts(hc, P)
                        pg = pg_pool.tile([P, TOK_TILE], f32, tag="pg")
                        pu = pu_pool.tile([P, TOK_TILE], f32, tag="pu")
                        for kc in range(KC):
                            mm(pg[:], wgl(kc)[:, hs], xtl(xt_sb, kc),
                               start=(kc == 0), stop=(kc == KC - 1))
                        for kc in range(KC):
                            mm(pu[:], wul(kc)[:, hs], xtl(xt_sb, kc),
                               start=(kc == 0), stop=(kc == KC - 1))
                        fuse_hc(hc, pg, pu, h_sb)
                # packed g/u tail: one matmul column block for both
                pgu = pg_pool.tile([P, TOK_TILE], f32, tag="pg")
                for kc in range(KC):
                    mm(pgu[:], wgu_sb[:, kc, :], xtl(xt_sb, kc),
                       start=(kc == 0), stop=(kc == KC - 1))
                tail_hc(pgu, h_sb)
                return h_sb, sc_sb

            def down(j, h_sc):
                h_sb, sc_sb = h_sc
                js = bass.ts(j, TOK_TILE)
                for dc in range(KC):
                    psum_y = py_pool.tile([P, TOK_TILE], f32, tag="py")
                    ds_ = bass.ts(dc, P)
                    for hc in range(HC):
                        kw = P if dt_name == "bf16" else min(P, H - hc * P)
                        mm(psum_y, wd_sb[:kw, hc, ds_], h_sb[:kw, hc, :],
                           start=(hc == 0), stop=(hc == HC - 1))
                    yt_sb = opool.tile([P, TOK_TILE], f32, tag="yt")
                    nc.vector.tensor_mul(yt_sb, psum_y, sc_sb)
                    # outputs go out on the scalar engine's HWDGE ring so
                    # they never queue behind input streaming
                    nc.scalar.dma_start(yt_d[dc][:, js], yt_sb[:])

            # software pipeline: down(j) is emitted after gate_up(j+1) so the
            # PE never waits on the DVE h-tiles
            h_prev = gate_up(0, xts[0])
            for j in range(1, n_tiles):
                h_cur = gate_up(j, xts[j] if j in xts else load_tile(j))
                down(j - 1, h_prev)
                h_prev = h_cur
            down(n_tiles - 1, h_prev)

    _split_excess_waits(nc, mybir, bass_rust)
    return nc


def _get_runner(C, dt_name):
    key = (C, dt_name)
    if key not in _RUNNERS:
        _RUNNERS[key] = _build(C, dt_name)
    return _RUNNERS[key]


def _routing(xt, Wg1, Wg2):
    logits = np.tanh(xt @ Wg1) @ Wg2  # [T, E] fp32
    order = np.argsort(-logits, axis=1, kind="stable")[:, :K]  # top-2, ties->low idx
    top_v = np.take_along_axis(logits, order, axis=1)
    # softmax over the selected 2
    m = top_v.max(axis=1, keepdims=True)
    ex = np.exp(top_v - m)
    top_s = ex / ex.sum(axis=1, keepdims=True)  # [T, 2]
    scores = np.zeros((T, E), dtype=np.float32)
    np.put_along_axis(scores, order, top_s.astype(np.float32), axis=1)
    return scores


def _cv_sq(v):
    v = v.astype(np.float64)
    return v.var(ddof=1) / (v.mean() ** 2 + 1e-10)


def _prepare(x, Wg1, Wg2, W_gate, W_up, W_down):
    """Routing + dispatch: returns (in_maps, idxs, C, balance_loss)."""
    x = np.asarray(x, dtype=np.float32)
    xt = x.reshape(T, D)
    scores = _routing(xt, np.asarray(Wg1, np.float32), np.asarray(Wg2, np.float32))

    importance = scores.sum(axis=0)
    load = (scores > 0).sum(axis=0).astype(np.float64)
    balance_loss = np.float32(BALANCE_W * (_cv_sq(importance) + _cv_sq(load)))

    idxs = [np.nonzero(scores[:, e] > 0)[0] for e in range(E)]
    max_cnt = max(len(i) for i in idxs)
    C = max(TOK_TILE, ((max_cnt + TOK_TILE - 1) // TOK_TILE) * TOK_TILE)

    np_io = ml_dtypes.bfloat16 if DT == "bf16" else np.float32
    W_gate = np.asarray(W_gate, np.float32)
    W_up = np.asarray(W_up, np.float32)
    W_down = np.asarray(W_down, np.float32)
    HF = H // P
    HB = HF * P

    in_maps = []
    for e in range(E):
        idx = idxs[e]
        # partition-major layouts: [P, KC, ...] so each tensor is one DMA
        xt_sel = np.zeros((KC, P, C), dtype=np_io)
        xt_sel.reshape(D, C)[:, : len(idx)] = xt[idx].T
        sc = np.zeros((1, C), dtype=np.float32)
        sc[0, : len(idx)] = scores[idx, e]
        wg3 = W_gate[e].astype(np_io).reshape(KC, P, H)
        wu3 = W_up[e].astype(np_io).reshape(KC, P, H)
        wd5 = np.zeros((HC, P, D), dtype=np_io)
        wd5.reshape(HC * P, D)[:H] = W_down[e].astype(np_io)
        wd4 = np.ascontiguousarray(wd5.transpose(1, 0, 2))
        w_all = np.concatenate(
            [
                wg3[:, :, :HB].transpose(1, 0, 2).reshape(P, -1),
                wu3[:, :, :HB].transpose(1, 0, 2).reshape(P, -1),
                np.concatenate([wg3[:, :, HB:], wu3[:, :, HB:]], axis=2)
                .transpose(1, 0, 2)
                .reshape(P, -1),
                wd4.reshape(P, -1),
            ],
            axis=1,
        )
        in_maps.append(
            {
                "xt": np.ascontiguousarray(xt_sel.transpose(1, 0, 2)),
                "w": np.ascontiguousarray(w_all),
                "sc": sc,
            }
        )
    return in_maps, idxs, C, balance_loss


def kernel(x, Wg1, Wg2, W_gate, W_up, W_down, num_selects):
    from concourse.bass_utils import run_bass_kernel_spmd

    assert int(num_selects) == K
    in_maps, idxs, C, balance_loss = _prepare(x, Wg1, Wg2, W_gate, W_up, W_down)

    nc = _get_runner(C, DT)
    res = run_bass_kernel_spmd(nc, in_maps, core_ids=list(range(N_CORES)))

    y = np.zeros((T, D), dtype=np.float32)
    for e in range(E):
        idx = idxs[e]
        yt = res.results[e]["yt"].reshape(D, C)
        y[idx] += yt[:, : len(idx)].T

    return y.reshape(np.asarray(x).shape[:-1] + (D,)), balance_loss
